# revision 17
# baseline (speedup 1.0000x reference)
"""Trainium2 Bass kernel for nn_EntityMapping (P=16 independent MLPs over a
shared entity batch).

Sharding: 16 partition-MLPs split across 8 NeuronCores (2 per core,
expert-parallel); the embedding batch is replicated.

Precision: fp8 e4m3 everywhere on the two big GEMMs, run in
MatmulPerfMode.DoubleRow (2 fp8 weights per PE cell -> K=256 per matmul at
0.5 cycles/row).  All quantization happens host-side with power-of-2 scales
chosen to keep every tensor inside e4m3's normal range: e*32, W*2048, h*16.
PSUM accumulates in fp32; elementwise engines fold descale + bias + relu
into the PSUM->SBUF eviction and emit the next layer's fp8 activations
directly.  End-to-end error ~3.2e-3 fro vs the fp32 reference (gate 2e-2).

Structure: entity chunks are processed in PAIRS (2 x 512 columns sharing a
2-bank PSUM tile) so that (a) each DoubleRow stationary weight is used by
two back-to-back matmuls, amortizing its (FWL-less) 256-column LDWEIGHTS,
and (b) each relu covers FD=1024, halving per-op overhead on ACT/DVE.  The
relu work is spread across ACT, DVE, and GPSIMD; the two MLPs' logits land
in one PSUM bank (partitions 0 and 32) so one sigmoid per chunk finishes
both.  L2 for a pair is issued after the next pair's L0 matmuls so the PE
never waits on relu latency.
"""

import os
import sys

import numpy as np

if "jax" not in sys.modules and os.environ.get("JAX_PLATFORMS") == "cpu":
    # don't let a cpu pin hide the axon/neuron backend the kernel runs on
    os.environ["JAX_PLATFORMS"] = ""

try:
    import concourse.bass as bass  # noqa: F401
except ImportError:  # harness runs kernel.py from a bare directory
    sys.path.insert(0, "/opt/trn_rl_repo")

import ml_dtypes

import concourse.mybir as mybir
import concourse.tile as tile
from concourse import bacc
from concourse.bass_utils import run_bass_kernel_spmd

F32 = mybir.dt.float32
F32R = mybir.dt.float32r
FP8 = mybir.dt.float8e4
E4NP = ml_dtypes.float8_e4m3  # TRN FP8_EXP4 (max normal 240)
RELU = mybir.ActivationFunctionType.Relu
SIGMOID = mybir.ActivationFunctionType.Sigmoid
DR = mybir.MatmulPerfMode.DoubleRow
DRS = mybir.MatmulPerfMode.DoubleRowSwInterleave
MULT = mybir.AluOpType.mult
ADD = mybir.AluOpType.add

P_TOTAL = 16  # independent MLP partitions
E = 512  # entity/embedding dim
H = 512  # hidden dim
N = 8192  # batch (entities)
N_CORES = 8
P_PER = P_TOTAL // N_CORES  # 2 MLPs per core
JC = H // 128  # 4 output-feature groups per layer
KT = E // 128  # 4 k-tiles of 128 (2 DoubleRow pairs)
NCH = 512  # batch columns per chunk (= PSUM bank in fp32)
NCHUNKS = N // NCH  # 16
NPAIRS = NCHUNKS // 2  # 8
SE = 32.0  # embedding fp8 scale
SW = 2048.0  # weight fp8 scale
SH = 16.0  # hidden-activation fp8 scale
SC_L0 = SH / (SE * SW)  # 2^-12: psum->h1q descale inside relu
SC_L1 = SH / (SH * SW)  # 2^-11
SC_L2 = 1.0 / (SH * SW)  # 2^-15: logit descale inside sigmoid
# relu engine assignment per layer: j-index -> "act" | "dve" | "gps"
RELU_ENG = {
    0: {0: "act", 1: "act", 2: "dve", 3: "dve"},
    1: {0: "act", 1: "act", 2: "act", 3: "dve"},
}
WARMUP_MM = 10  # junk matmuls to warm the PE clock during the load window


def _build():
    nc = bacc.Bacc(
        "TRN2", target_bir_lowering=False, debug=False, num_devices=N_CORES
    )
    # All inputs pre-packed + pre-quantized on host into SBUF layout.
    et_dram = nc.dram_tensor("et8", [128, KT * N], FP8, kind="ExternalInput")
    w0_dram = nc.dram_tensor(
        "w0", [128, P_PER * JC * KT * 128], FP8, kind="ExternalInput"
    )
    w1_dram = nc.dram_tensor(
        "w1", [128, P_PER * JC * KT * 128], FP8, kind="ExternalInput"
    )
    w2_dram = nc.dram_tensor("w2", [128, P_PER * KT * 64], FP8, kind="ExternalInput")
    b0_dram = nc.dram_tensor("b0", [128, P_PER * JC], F32, kind="ExternalInput")
    b1_dram = nc.dram_tensor("b1", [128, P_PER * JC], F32, kind="ExternalInput")
    b2_dram = nc.dram_tensor("b2", [128, 1], F32, kind="ExternalInput")
    out_dram = nc.dram_tensor("out", [P_PER, N], F32, kind="ExternalOutput")

    et_v = et_dram.rearrange("p (g n) -> p g n", g=KT)  # [128, 4, N]
    w0_v = w0_dram.rearrange("q (p j i f) -> q p j i f", p=P_PER, j=JC, i=2)
    w1_v = w1_dram.rearrange("q (p j i f) -> q p j i f", p=P_PER, j=JC, i=2)
    w2_v = w2_dram.rearrange("q (p t s) -> q p t s", p=P_PER, t=KT)

    with tile.TileContext(nc) as tc:
        with (
            tc.tile_pool(name="const", bufs=1) as const_pool,
            tc.tile_pool(name="warm", bufs=1) as warm_pool,
            tc.tile_pool(name="h1", bufs=2) as h1_pool,
            tc.tile_pool(name="h2", bufs=2) as h2_pool,
            tc.tile_pool(name="osb", bufs=4) as out_pool,
            tc.tile_pool(name="mmps", bufs=3, space="PSUM") as ps_mm,
            tc.tile_pool(name="l2ps", bufs=1, space="PSUM") as ps_l2,
        ):
            # persistent fp8 inputs
            et_sb = const_pool.tile([128, KT, N], FP8, tag="et")
            w0_sb = const_pool.tile([128, P_PER, JC, 2, 256], FP8, tag="w0")
            w1_sb = const_pool.tile([128, P_PER, JC, 2, 256], FP8, tag="w1")
            w2_sb = const_pool.tile([128, P_PER, KT, 64], FP8, tag="w2")
            b0_sb = const_pool.tile([128, P_PER * JC], F32, tag="b0")
            b1_sb = const_pool.tile([128, P_PER * JC], F32, tag="b1")
            b2_sb = const_pool.tile([128, 1], F32, tag="b2")

            # two persistent logit PSUM banks (pair-alternating); each holds
            # MLP0's logits at partition 0 and MLP1's at partition 32 so a
            # single sigmoid ACT op finishes a whole chunk
            r_ab = [
                ps_l2.tile([128, NCH], F32, tag=f"r{x}", name=f"r{x}")
                for x in range(2)
            ]

            # --- PE warmup: junk matmuls on a memset tile so the HAM clock
            # is at K=8/8 when the first real matmul issues ---
            wm_f = warm_pool.tile([128, 640], F32, tag="wmf")
            nc.gpsimd.memset(wm_f[:], 0.0)
            wm_r = warm_pool.tile([128, 640], F32R, tag="wmr")
            nc.vector.tensor_copy(wm_r[:], wm_f[:])
            for i in range(WARMUP_MM):
                nc.tensor.matmul(
                    r_ab[0][:],
                    wm_r[:, 0:128],
                    wm_r[:, 128:640],
                    start=(i == 0),
                    stop=(i == WARMUP_MM - 1),
                )

            # --- input streaming, in consumption order ---
            def load_et(c, eng=None):
                n0 = c * NCH
                (eng or nc.sync).dma_start(
                    et_sb[:, :, n0 : n0 + NCH], et_v[:, :, n0 : n0 + NCH]
                )

            load_et(0, eng=nc.gpsimd)  # rides the gpsimd queue: issues now
            load_et(1, eng=nc.gpsimd)
            nc.sync.dma_start(b0_sb[:], b0_dram[:])
            nc.sync.dma_start(b1_sb[:], b1_dram[:])
            nc.sync.dma_start(b2_sb[:], b2_dram[:])
            nc.sync.dma_start(w2_sb[:], w2_v[:])
            for p in range(P_PER):
                for w_sb, w_v in ((w0_sb, w0_v), (w1_sb, w1_v)):
                    for jh in range(2):
                        nc.sync.dma_start(
                            w_sb[:, p, 2 * jh : 2 * jh + 2],
                            w_v[:, p, 2 * jh : 2 * jh + 2],
                        )
            load_et(2)
            load_et(3)

            def relu_emit(ps, dst, scale, bias_col, eng):
                # dst = fp8(relu(ps * scale + bias)), over a [128,2,NCH] pair
                if eng == "act":
                    nc.scalar.activation(dst, ps[:], RELU, bias=bias_col, scale=scale)
                else:
                    e = nc.vector if eng == "dve" else nc.gpsimd
                    e.tensor_scalar(dst, ps[:], scale, bias_col, MULT, ADD)
                    e.tensor_scalar_max(dst, dst, 0.0)

            def layer(li, p, w_sb, mv, h_pool, scale, b_sb, eng_map=None):
                # one 512->512 fp8 DoubleRow layer for MLP p on a chunk pair;
                # mv(i, cx) gives the [128, 2, NCH] moving operand of k-pair
                # i for pair half cx.  Each stationary weight is used by two
                # consecutive matmuls (the two halves).
                h = h_pool.tile([128, KT, 2, NCH], FP8, tag="h")
                for j in range(JC):
                    ps = ps_mm.tile([128, 2, NCH], F32, tag="mm")
                    for i in range(2):
                        for cx in range(2):
                            nc.tensor.matmul(
                                ps[:, cx, :],
                                w_sb[:, p, j, i, :],
                                mv(i, cx),
                                start=(i == 0),
                                stop=(i == 1),
                                perf_mode=DRS,
                            )
                    bias_col = b_sb[:, p * JC + j : p * JC + j + 1]
                    eng = (eng_map or RELU_ENG[li])[j]
                    relu_emit(ps, h[:, j, :, :], scale, bias_col, eng)
                return h

            def l2(cp, h2s):
                # logits for both MLPs of pair cp; h2s[p] is that MLP's
                # [128, KT, 2, NCH] hidden tile
                r = r_ab
                # p1 runs at M=64 (w2 in stationary column 32, zeros elsewhere)
                # so its logit lands at PSUM partition 32 with tile_position
                # (0,0) -- DoubleRow forbids column-offset tile positions --
                # and the zero columns define partitions 0..63; p0 (M=1) then
                # overwrites partition 0 with its own start-group.
                for p, msl in ((1, slice(0, 64)), (0, slice(0, 1))):
                    for i in range(2):
                        for cx in range(2):
                            nc.tensor.matmul(
                                r[cx][msl, :],
                                w2_sb[:, p, 2 * i : 2 * i + 2, msl],
                                h2s[p][:, 2 * i : 2 * i + 2, cx, :],
                                start=(i == 0),
                                stop=(i == 1),
                                perf_mode=DR,
                            )
                for cx in range(2):
                    n0 = (2 * cp + cx) * NCH
                    o = out_pool.tile([64, NCH], F32, tag="o")
                    nc.scalar.activation(
                        o[:], r[cx][0:64, :], SIGMOID, bias=b2_sb[0:64, :],
                        scale=SC_L2,
                    )
                    for p in range(P_PER):
                        nc.sync.dma_start(
                            out_dram[p : p + 1, n0 : n0 + NCH],
                            o[32 * p : 32 * p + 1, :],
                        )

            prev = None
            for cp in range(NPAIRS):
                for c in (2 * cp + 4, 2 * cp + 5):
                    if c < NCHUNKS:
                        load_et(c)
                na, nb = 2 * cp * NCH, (2 * cp + 1) * NCH

                def et_mv(i, cx, _n=(na, nb)):
                    n0 = _n[cx]
                    return et_sb[:, 2 * i : 2 * i + 2, n0 : n0 + NCH]

                if cp == 0:
                    # ramp: p-major so compute starts after only p0's weights
                    h2s = []
                    for p in range(P_PER):
                        h1 = layer(0, p, w0_sb, et_mv, h1_pool, SC_L0, b0_sb)

                        def h1_mv(i, cx, _h=h1):
                            return _h[:, 2 * i : 2 * i + 2, cx, :]

                        h2s.append(
                            layer(1, p, w1_sb, h1_mv, h2_pool, SC_L1, b1_sb)
                        )
                    prev = (cp, h2s)
                    continue
                h1s = [
                    layer(0, p, w0_sb, et_mv, h1_pool, SC_L0, b0_sb)
                    for p in range(P_PER)
                ]
                if prev is not None:
                    l2(*prev)  # previous pair's logits: relus are long done
                h2s = []
                last = cp == NPAIRS - 1
                for p in range(P_PER):
                    h1 = h1s[p]

                    def h1_mv(i, cx, _h=h1):
                        return _h[:, 2 * i : 2 * i + 2, cx, :]

                    eng = {0: "act", 1: "dve", 2: "dve", 3: "act"} if last else None
                    h2s.append(
                        layer(1, p, w1_sb, h1_mv, h2_pool, SC_L1, b1_sb, eng)
                    )
                prev = (cp, h2s)
            l2(*prev)

    if os.environ.get("K_DEDUP_LDW") == "1":
        # hw experiment: NRT_EXEC_UNIT_UNRECOVERABLE — the PE appears to
        # need its LDWEIGHTS before every DoubleRow matmul
        _dedup_ldweights(nc)
    nc.compile()
    return nc


def _dedup_ldweights(nc):
    """Remove InstLdweights whose stationary operand is identical to the
    weights already sitting in the PE array (loaded by the immediately
    preceding InstLdweights, with only non-self-loading Matmults between).
    The chunk-pair loop issues every DoubleRow weight twice back-to-back, so
    this halves the (FWL-less, 256-column) weight-load traffic that
    otherwise serializes with the matmul stream."""
    for blk in nc.m.functions[0].blocks:
        insts = list(blk.instructions)
        new_list = []
        last_sig = None
        pend_waits, pend_updates = [], []
        for inst in insts:
            op = inst.concise_opcode()
            if op == "Ldweights":
                a = inst.ins[0]
                sig = (
                    str(a.memref), str(a.ap), a.offset,
                    str(inst.perf_mode), str(inst.tile_position),
                    str(inst.tile_size), bool(inst.is_transpose),
                )
                if sig == last_sig:  # redundant reload: drop, carry its sync
                    si = inst.sync_info
                    if si is not None:
                        pend_waits.extend(si.on_wait)
                        pend_updates.extend(si.on_update)
                    continue
                last_sig = sig
            elif op == "Matmult":
                if pend_waits or pend_updates:
                    si = inst.sync_info
                    if si is None:
                        inst.sync_info = mybir.SyncInfo(
                            on_wait=list(pend_waits), on_update=list(pend_updates)
                        )
                    else:
                        si.on_wait = list(si.on_wait) + pend_waits
                        si.on_update = list(si.on_update) + pend_updates
                    pend_waits, pend_updates = [], []
                if inst.ldweights:  # self-loading matmul clobbers the array
                    last_sig = None
            elif op in ("br", "UnconditionalBranch", "Call"):
                last_sig = None
            new_list.append(inst)
        assert not pend_waits and not pend_updates
        if len(new_list) != len(insts):
            while len(blk.instructions):
                blk.instructions.pop()
            for inst in new_list:
                blk.instructions.append(inst)


_NC_CACHE = None


def _get_nc():
    global _NC_CACHE
    if _NC_CACHE is None:
        _NC_CACHE = _build()
    return _NC_CACHE


def _q8(x, scale):
    return np.clip(x * scale, -240.0, 240.0).astype(np.float32).astype(E4NP)


def _make_in_maps(e_embedding, W0, b0, W1, b1, W2, b2):
    e = np.asarray(e_embedding, dtype=np.float32)
    W0 = np.asarray(W0, dtype=np.float32)
    b0 = np.asarray(b0, dtype=np.float32)
    W1 = np.asarray(W1, dtype=np.float32)
    b1 = np.asarray(b1, dtype=np.float32)
    W2 = np.asarray(W2, dtype=np.float32)
    b2 = np.asarray(b2, dtype=np.float32)

    # [E, N] -> [g, 128, N] -> [128, g, N]: partition = k%128, g = k//128
    et8 = np.ascontiguousarray(
        _q8(e.T, SE).reshape(KT, 128, N).transpose(1, 0, 2)
    ).reshape(128, KT * N)

    def pack_w(W, sl):  # [2, 512, 512] -> [128, (p j i f)] SwInterleaved
        q = _q8(W[sl], SW)  # [P_PER, K, M]
        q = q.reshape(P_PER, 2, 2, 128, JC, 128)  # [p, i, t2, part, j, m]
        q = q.transpose(3, 0, 4, 1, 5, 2)  # [part, p, j, i, m, t2]
        # hw reads column pairs (A_m, B_m) interleaved, columns reversed
        q = q[:, :, :, :, ::-1, :]  # reverse m
        return np.ascontiguousarray(q).reshape(128, -1)

    in_maps = []
    for cid in range(N_CORES):
        sl = slice(P_PER * cid, P_PER * (cid + 1))
        w2q = np.zeros((128, P_PER, KT, 64), dtype=E4NP)
        w2cols = _q8(W2[sl, :, 0], SW).reshape(P_PER, KT, 128).transpose(2, 0, 1)
        w2q[:, 0, :, 0] = w2cols[:, 0]
        w2q[:, 1, :, 32] = w2cols[:, 1]
        b0t = np.ascontiguousarray(
            (SH * b0[sl]).reshape(P_PER, JC, 128).transpose(2, 0, 1).reshape(128, -1)
        )
        b1t = np.ascontiguousarray(
            (SH * b1[sl]).reshape(P_PER, JC, 128).transpose(2, 0, 1).reshape(128, -1)
        )
        b2t = np.zeros((128, 1), dtype=np.float32)
        b2t[0, 0] = b2[sl][0, 0]
        b2t[32, 0] = b2[sl][1, 0]
        in_maps.append(
            {
                "et8": et8,
                "w0": pack_w(W0, sl),
                "w1": pack_w(W1, sl),
                "w2": w2q.reshape(128, -1),
                "b0": b0t,
                "b1": b1t,
                "b2": b2t,
            }
        )
    return in_maps


def kernel_with_results(trace=False, **inputs):
    nc = _get_nc()
    in_maps = _make_in_maps(**inputs)
    try:
        res = run_bass_kernel_spmd(
            nc, in_maps, core_ids=list(range(N_CORES)), trace=trace
        )
    except Exception:
        # the first PJRT compile in a fresh container can fail transiently;
        # one retry reuses the primed NEFF cache
        res = run_bass_kernel_spmd(
            nc, in_maps, core_ids=list(range(N_CORES)), trace=trace
        )
    full = np.concatenate([r["out"] for r in res.results], axis=0)  # [16, N]
    out = np.ascontiguousarray(full.T).astype(np.float32)  # [N, 16]
    return out, res


def kernel(**inputs):
    out, _ = kernel_with_results(trace=False, **inputs)
    return out
